# revision 37
# baseline (speedup 1.0000x reference)
"""HDTimeCrystalBlock kernel for 8 Trainium2 NeuronCores.

Math: out = ((x @ W_in) * mod[None]) @ W_out, where
  mod[l,h] = sum_m coupled[m] * cos(omega*(m+1)*t[l] + E[m,h])
Using cos(a+b) = cos(a)cos(b) - sin(a)sin(b):
  mod = C @ A + S @ B,  A[m,h] = coupled[m]*cos(E[m,h]),
                        B[m,h] = -coupled[m]*sin(E[m,h]),
  C[l,m] = cos(omega*(m+1)*t[l]), S[l,m] = sin(...)
so the [L,M,HD] cos tensor never materializes: mod is a K=2M matmul.
A/B and C/S are tiny trig tables computed on the host (like cs was
before); the device only does matmuls + the elementwise modulate.

Sharding: split L=2048 into 8 chunks of 256; each core handles its
l-chunk for ALL 4 batches (1024 tokens, laid out b-major). mod depends
only on l, so each core computes each mod tile ONCE ([128,256] per
h-tile, cached in SBUF) and reuses it across the 4 batch blocks -- 4x
less mod-matmul work than batch-sharding. Activations stay transposed
([feature, token]); weights are the stationary operand. Matmuls in
bf16. DMAs are ordered so the PE starts within ~3us and never starves.
"""
import math

import numpy as np

B, L, D, HD, M = 4, 2048, 512, 4096, 16
NCORES = 8
LCH = L // NCORES              # l-chunk per core (256)
T = B * LCH                    # tokens per core (1024), b-major
QCH = 512                      # token-chunk (PSUM bank width in fp32)
NQ = T // QCH                  # 2
NJ = HD // 128                 # 32 h-tiles
NK = D // 128                  # 4 d-tiles
CW = 4                         # w_in/w_out column chunks (1024 cols each)
JPC = NJ // CW                 # 8 j-tiles per chunk

_cache = {}


def _build():
    from concourse import bacc, bass, mybir, tile

    F32 = mybir.dt.float32
    BF16 = mybir.dt.bfloat16
    PSUM = bass.MemorySpace.PSUM

    nc = bacc.Bacc("TRN2", target_bir_lowering=False, debug=False)

    xT_d = nc.dram_tensor("xT", [D, T], BF16, kind="ExternalInput")
    w_in_d = nc.dram_tensor("w_in", [D, HD], BF16, kind="ExternalInput")
    w_out_d = nc.dram_tensor("w_out", [HD, D], BF16, kind="ExternalInput")
    # csab packs cs ([2M, LCH] trig of omega*m*t) and ab ([2M, HD]
    # coupled*cos/sin(E)) into one DMA
    csab_d = nc.dram_tensor("csab", [2 * M, LCH + HD], BF16, kind="ExternalInput")
    yT_d = nc.dram_tensor("yT", [D, T], BF16, kind="ExternalOutput")

    with tile.TileContext(nc) as tc:
        with (
            tc.tile_pool(name="win", bufs=1) as winp,
            tc.tile_pool(name="wout", bufs=1) as woutp,
            tc.tile_pool(name="xts", bufs=1) as xtp,
            tc.tile_pool(name="small", bufs=1) as smallp,
            tc.tile_pool(name="hm", bufs=4) as hmp,
            tc.tile_pool(name="yo", bufs=4) as yop,
            tc.tile_pool(name="pa", bufs=3, space=PSUM) as pap,
            tc.tile_pool(name="pb", bufs=1, space=PSUM) as pbp,
            tc.tile_pool(name="py", bufs=4, space=PSUM) as pyp,
        ):
            # ---- bulk loads: all on the sync ring (the scalar engine's
            # FIFO must stay free for the msb/yo copies -- DMA issues
            # pace on only 8 semaphore lanes and would block them).
            # Ordered by first use so the PE starts early, never starves.
            # xts0 rides the scalar (Act) ring -- a single issue, done
            # long before the first msb copy needs the scalar engine -- so
            # the head-of-stream bytes split across both HWDGE rings.
            # wm is memset (no DMA dependency): the warm-up matmuls can
            # start the moment the engine preamble ends and bring HAM to
            # 8/8 before real work arrives.
            wm = smallp.tile([128, 384], BF16, tag="wm")
            nc.vector.memset(wm[:], 1.0)

            w_in_r = w_in_d.ap().rearrange("(k p) (c h) -> c p k h", p=128, c=CW)
            xT_r = xT_d.ap().rearrange("(k p) (q t) -> q p k t", p=128, q=NQ)
            w_out_r = w_out_d.ap().rearrange(
                "(g jj p) i -> g p jj i", p=128, jj=JPC
            )
            yT_r = yT_d.ap().rearrange("(j2 p) (q t) -> q p j2 t", p=128, q=NQ)

            win_c = [None] * CW
            xts_q = [None] * NQ
            wout_g = [None] * CW

            def load_win(c):
                t_ = winp.tile([128, NK, 1024], BF16, name=f"win{c}", tag=f"win{c}")
                nc.sync.dma_start(t_[:], w_in_r[c])
                win_c[c] = t_

            def load_xts(q, eng=None):
                tx = xtp.tile([128, NK, QCH], BF16, name=f"xts{q}", tag=f"xts{q}")
                (eng or nc.sync).dma_start(tx[:], xT_r[q])
                xts_q[q] = tx

            def load_wout(g):
                tw = woutp.tile([128, JPC, D], BF16, name=f"wout{g}", tag=f"wout{g}")
                nc.sync.dma_start(tw[:], w_out_r[g])
                wout_g[g] = tw

            # csab padded to K=128 (rows 2M:128 zero) so the pb matmuls
            # share the full row-group config with pa/py -- mixing K=32
            # and K=128 matmuls costs ~96ns per row-group switch
            csab = smallp.tile([128, LCH + HD], BF16, tag="csab")
            for zp in range(2 * M, 128, 32):
                nc.gpsimd.memset(csab[zp : zp + 32, :], 0.0)
            # xts0 in two k-halves on the scalar ring; win0's j0 strip
            # first on the sync ring: the first pa group can start ~2us
            # sooner than waiting for whole-tile transfers
            tx0 = xtp.tile([128, NK, QCH], BF16, name="xts0", tag="xts0")
            nc.scalar.dma_start(tx0[:, 0:2, :], xT_r[0][:, 0:2, :])
            nc.sync.dma_start(tx0[:, 2:NK, :], xT_r[0][:, 2:NK, :])
            xts_q[0] = tx0
            w0 = winp.tile([128, NK, 1024], BF16, name="win0", tag="win0")
            nc.sync.dma_start(w0[:, :, 0:128], w_in_r[0][:, :, 0:128])
            nc.sync.dma_start(csab[0 : 2 * M, :], csab_d[:])
            nc.sync.dma_start(w0[:, :, 128:512], w_in_r[0][:, :, 128:512])
            win_c[0] = w0
            wo0 = woutp.tile([128, JPC, D], BF16, name="wout0", tag="wout0")
            nc.sync.dma_start(wo0[:, 0:2, :], w_out_r[0][:, 0:2, :])
            nc.sync.dma_start(w0[:, :, 512:1024], w_in_r[0][:, :, 512:1024])
            nc.sync.dma_start(wo0[:, 2:JPC, :], w_out_r[0][:, 2:JPC, :])
            wout_g[0] = wo0
            load_win(1)
            load_wout(1)
            load_xts(1)
            for c in range(2, CW):
                load_win(c)
                load_wout(c)

            # ---- PE warm-up: scratch matmuls on garbage keep the PE busy
            # (and HAM warming) from preamble-end until the real stream ----
            for w in range(32):
                pw = pap.tile([128, LCH], F32, name=f"warm{w}", tag="pa")
                nc.tensor.matmul(
                    pw[:], wm[:, 0:128], wm[:, 128:384], start=True, stop=True
                )

            # mod cache: msb[:, 256*j:256*(j+1)] holds mod h-tile j
            msb = smallp.tile([128, NJ * LCH], BF16, tag="msb")

            # both pb slots live in one PSUM bank (subtile-tracked halves)
            pbt = pbp.tile([128, 2, LCH], F32, tag="pb", name="pbt")

            def emit_pb(j):
                pb = pbt[:, j % 2, :]
                nc.tensor.matmul(
                    pb,
                    csab[:, LCH + 128 * j : LCH + 128 * (j + 1)],
                    csab[:, 0:LCH],
                    start=True,
                    stop=True,
                )
                ms = msb[:, LCH * j : LCH * (j + 1)]
                # copies alternate scalar/vector so pb's PSUM recycling
                # never gates the PE
                if j % 2 == 0:
                    nc.scalar.copy(ms, pb)
                else:
                    nc.vector.tensor_copy(ms, pb)

            # the first pb (mod) matmuls only need csab: they fill the
            # rest of the DMA-wait window with real work
            NPRE = 10
            for j in range(NPRE):
                emit_pb(j)

            # ---- fused main loop (py stage software-pipelined by two j,
            # so PE never waits on the vector-engine modulate) ----
            for q in range(NQ):
                lo, hi = q * QCH, (q + 1) * QCH
                pys = [pyp.tile([128, QCH], F32, name=f"py{q}_{j2}", tag="py")
                       for j2 in range(NK)]

                def emit_py(phm, pj):
                    for j2 in range(NK):
                        nc.tensor.matmul(
                            pys[j2][:],
                            wout_g[pj // JPC][:, pj % JPC,
                                              128 * j2 : 128 * (j2 + 1)],
                            phm[:],
                            start=(pj == 0),
                            stop=(pj == NJ - 1),
                        )

                pend = []
                for j in range(NJ):
                    c, jc = j // JPC, j % JPC
                    pa = pap.tile([128, QCH], F32, tag="pa")
                    for k in range(NK):
                        nc.tensor.matmul(
                            pa[:],
                            win_c[c][:, k, 128 * jc : 128 * (jc + 1)],
                            xts_q[q][:, k, :],
                            start=(k == 0),
                            stop=(k == NK - 1),
                        )
                    ms = msb[:, LCH * j : LCH * (j + 1)]
                    if q == 0 and j >= NPRE:
                        emit_pb(j)
                    hm = hmp.tile([128, QCH], BF16, tag="hm")
                    nc.vector.tensor_mul(hm[:, 0:LCH], pa[:, 0:LCH], ms)
                    nc.vector.tensor_mul(hm[:, LCH:QCH], pa[:, LCH:QCH], ms)
                    pend.append((hm, j))
                    if len(pend) > 2:
                        emit_py(*pend.pop(0))
                for phm, pj in pend:
                    emit_py(phm, pj)
                # evictions alternate scalar/vector into one contiguous
                # buffer; the store leaves split across both HWDGE rings
                yo = yop.tile([128, NK, QCH], BF16, tag="yo")
                for j2 in range(NK):
                    if j2 % 2 == 0:
                        nc.scalar.copy(yo[:, j2, :], pys[j2][:])
                    else:
                        nc.vector.tensor_copy(yo[:, j2, :], pys[j2][:])
                nc.sync.dma_start(yT_r[q][:, 0:1, :], yo[:, 0:1, :])
                nc.scalar.dma_start(yT_r[q][:, 1:2, :], yo[:, 1:2, :])
                nc.sync.dma_start(yT_r[q][:, 2:3, :], yo[:, 2:3, :])
                nc.scalar.dma_start(yT_r[q][:, 3:4, :], yo[:, 3:4, :])

    nc.finalize()
    return nc


def _get_nc():
    if "nc" not in _cache:
        _cache["nc"] = _build()
    return _cache["nc"]


def _bf(a):
    import ml_dtypes
    return np.ascontiguousarray(np.asarray(a, dtype=np.float32).astype(ml_dtypes.bfloat16))


def _in_maps(x, input_proj, output_proj, floquet_energies, drive_weights,
             coupling_matrix):
    coupled = coupling_matrix.astype(np.float64) @ drive_weights.astype(np.float64)
    E = floquet_energies.astype(np.float64)
    ab_np = np.empty((2 * M, HD), dtype=np.float64)
    ab_np[0:M] = coupled[:, None] * np.cos(E)
    ab_np[M:] = -coupled[:, None] * np.sin(E)
    w_in = _bf(input_proj)
    w_out = _bf(output_proj)

    harm = np.arange(1, M + 1, dtype=np.float64)
    maps = []
    for c in range(NCORES):
        t = (c * LCH + np.arange(LCH, dtype=np.float64)) / L
        ang = 2.0 * np.pi * harm[:, None] * t[None, :]
        csab_np = np.empty((2 * M, LCH + HD), dtype=np.float64)
        csab_np[0:M, 0:LCH] = np.cos(ang)
        csab_np[M:, 0:LCH] = np.sin(ang)
        csab_np[:, LCH:] = ab_np
        # xT[d, b*LCH + l] = x[b, c*LCH + l, d]
        xc = x[:, c * LCH : (c + 1) * LCH, :]          # [B, LCH, D]
        xT = _bf(xc.transpose(2, 0, 1).reshape(D, T))
        maps.append({
            "xT": xT,
            "w_in": w_in,
            "w_out": w_out,
            "csab": _bf(csab_np),
        })
    return maps


def kernel(x, input_proj, output_proj, floquet_energies, drive_weights,
           coupling_matrix, _trace=False, _trace_kwargs=None):
    from concourse.bass_utils import run_bass_kernel_spmd

    nc = _get_nc()
    maps = _in_maps(x, input_proj, output_proj, floquet_energies,
                    drive_weights, coupling_matrix)
    kw = dict(_trace_kwargs or {})
    res = run_bass_kernel_spmd(nc, maps, list(range(NCORES)), trace=_trace, **kw)
    out = np.empty((B, L, D), dtype=np.float32)
    for c in range(NCORES):
        yT = np.asarray(res.results[c]["yT"], dtype=np.float32)  # [D, T]
        out[:, c * LCH : (c + 1) * LCH, :] = yT.reshape(D, B, LCH).transpose(1, 2, 0)
    if _trace:
        return out, res
    return out
